# revision 11
# baseline (speedup 1.0000x reference)
"""Trainium2 Bass kernel for NeighborhoodAggregationEmbedding.

Math (reference):
  rel features per pair (i,j): dist, cos, sin, dx/(dist+eps), dy/(dist+eps), log1p(dist)
  kv = feats @ kv_w + kv_b ; k,v heads ; logits = q.k/sqrt(D); softmax over j
  (self-masked, pad-masked); ctx = attn.v ; MLP: LN(ctx@w1+b1) -> gelu -> @w2+b2

Key algebraic restructure (host-side, exact up to ~1e-7):
  * cos ~= dx/dist, sin ~= dy/dist so the 6 features collapse to 4:
    F = [dist, cx, cy, log1p(dist)].
  * query is shared by every (b, i) so logits = F @ A with a host-computed
    (4,4) matrix A; the cx/cy logit terms become (w[j]-w[i])*inv with
    w = a1*px + a2*py per node (padding folds into w[j] as -1e20).
  * attn.v  ==>  S[i,h,p] = sum_j E_h * F_p ; ctx = (S/Z) @ Wv16.
  * self-mask via analytic diagonal corrections on Z and S.
  * |logits| < ~1 for this input distribution (A ~ 1e-3), so bf16
    intermediates after the logit are safe.

Device strategy (v4):
  * "exp-replay": for F in {cx,cy,ld}, sum_j E*F = sum_j exp(l2 + ln F')
    computed on the (otherwise idle) scalar engine: DVE does one cheap
    bf16 2x add (l2b + lnF'b), Act does exp with accumulate. F' is
    range-shifted/scaled so ln F' is small where terms matter
    (cx+2, ld/4); scales fold into Wv16 / diag corrections host-side.
    Only the dist products stay as DVE fused multiply-accumulates.
  * inv = reciprocal_approx_fast (5x faster than exact reciprocal).
  * PX/PY broadcast to 128 partitions via chunked HBM DMAs split across
    the two HWDGE queues; WR/tail constants via gpsimd partition_broadcast
    (gpsimd never runs concurrently with DVE compute - SBUF contention
    halves DVE throughput).
  * activation-table switches minimized (Sqrt preload; Ln/Exp blocks).
  * gelu via exact-erf Gelu activation; LN gamma/beta and biases skipped
    on device when the host detects identity/zero values.

Per-core work (8 cores): core c -> batch b=c//2, query rows i in
[256*(c%2), 256*(c%2)+256); two [128 i x 512 j] tiles.
"""

import numpy as np

B, N, E, H = 4, 512, 128, 4
D = E // H
EPS = 1e-8
LN_EPS = 1e-5
BIG = 1e20
NCORES = 8

_f32 = np.float32

LD_SCALE = 0.25          # replay plane: ln(ld * LD_SCALE)
CX_BIAS = 2.0            # replay plane: ln(cx + 2)
REPLAY_LD = True
REPLAY_CXCY = True


def _host_prep(positions, key_padding_mask, kv_w, kv_b, query, w1, b1, ln_g, ln_b, w2, b2):
    pos = np.asarray(positions, dtype=_f32)
    pad = np.asarray(key_padding_mask).astype(bool)
    kv_w = np.asarray(kv_w, dtype=_f32)
    kv_b = np.asarray(kv_b, dtype=_f32)
    q = np.asarray(query, dtype=_f32).reshape(H, D)
    w1 = np.asarray(w1, dtype=_f32)
    b1 = np.asarray(b1, dtype=_f32)
    ln_g = np.asarray(ln_g, dtype=_f32)
    ln_b = np.asarray(ln_b, dtype=_f32)
    w2 = np.asarray(w2, dtype=_f32)
    b2 = np.asarray(b2, dtype=_f32)

    Wk = kv_w[:, :E]
    Wv = kv_w[:, E:]
    Wk4 = np.stack([Wk[0], Wk[1] + Wk[3], Wk[2] + Wk[4], Wk[5]]).astype(_f32)
    Wv4 = np.stack([Wv[0], Wv[1] + Wv[3], Wv[2] + Wv[4], Wv[5]]).astype(_f32)

    A = np.einsum("phd,hd->ph", Wk4.reshape(4, H, D), q) / np.sqrt(_f32(D))
    A = A.astype(_f32)

    b1_eff = (b1 + kv_b[E:] @ w1).astype(_f32)
    skip_b1 = bool(np.all(np.abs(b1_eff) < 1e-12))
    skip_ln = bool(np.all(ln_g == 1.0) and np.all(ln_b == 0.0))
    skip_b2 = bool(np.all(b2 == 0.0))

    wrow_nopad = (
        A[1][None, :, None] * pos[:, None, :, 0] + A[2][None, :, None] * pos[:, None, :, 1]
    ).astype(_f32)
    wrow = (wrow_nopad - _f32(BIG) * pad[:, None, :].astype(_f32)).astype(_f32)

    # analytic device diagonal values
    d0 = _f32(np.sqrt(_f32(EPS)))
    ld0 = _f32(np.log(_f32(1.0) + d0))
    e_diag = np.exp((A[0] * d0 + A[3] * ld0).astype(_f32)).astype(_f32)
    zcorr = e_diag.copy()
    # scorr is in ACCUMULATOR units per column (replay columns accumulate
    # scaled quantities). For replayed cx/cy columns S = R - 2*Z_all =
    # R - 2*Zc - 2*zcorr; the -2*Zc part is runtime, 2*zcorr is static here.
    scorr = np.zeros(16, dtype=_f32)
    for h in range(H):
        scorr[h * 4 + 0] = e_diag[h] * d0
        if REPLAY_CXCY:
            scorr[h * 4 + 1] = 2.0 * zcorr[h]
            scorr[h * 4 + 2] = 2.0 * zcorr[h]
        if REPLAY_LD:
            scorr[h * 4 + 3] = e_diag[h] * ld0 * LD_SCALE
        else:
            scorr[h * 4 + 3] = e_diag[h] * ld0
    scorr = scorr.astype(_f32)

    # Wv16[(h,p), e] = Wv4[p, e] restricted to head-h block; replayed ld
    # columns accumulate E*ld*LD_SCALE so those rows get rescaled.
    Wv16 = np.zeros((16, E), dtype=_f32)
    for h in range(H):
        for p in range(4):
            r = Wv4[p, h * D : (h + 1) * D]
            if p == 3 and REPLAY_LD:
                r = r / _f32(LD_SCALE)
            Wv16[h * 4 + p, h * D : (h + 1) * D] = r

    tailrow = np.concatenate([zcorr, scorr])[None, :].astype(_f32)  # [1, 20]

    shared = {
        "wv16": Wv16.astype(_f32),
        "w1": w1,
        "w2": w2,
        "tailrow": tailrow,
    }
    per_core = []
    for c in range(NCORES):
        b = c // 2
        i0 = (c % 2) * 256
        rowflat = np.concatenate([pos[b, :, 0], pos[b, :, 1], wrow[b].reshape(-1)])[None, :]
        colcat = np.concatenate(
            [pos[b, i0 : i0 + 256], wrow_nopad[b, :, i0 : i0 + 256].T], axis=1
        )
        per_core.append(
            {
                "rowflat": np.ascontiguousarray(rowflat, dtype=_f32),
                "colcat": np.ascontiguousarray(colcat, dtype=_f32),
                **shared,
            }
        )
    flags = {"skip_b1": skip_b1, "skip_ln": skip_ln, "skip_b2": skip_b2}
    if not (skip_b1 and skip_ln and skip_b2):
        extra = np.concatenate([b1_eff, ln_g, ln_b, b2])[None, :].astype(_f32)
        for pc in per_core:
            pc["extrarow"] = extra
    return per_core, A, flags


def _build_program(A, flags):
    import concourse.bacc as bacc
    import concourse.bass as bass
    import concourse.tile as tile
    from concourse import mybir
    from concourse.masks import make_identity

    f32 = mybir.dt.float32
    bf16 = mybir.dt.bfloat16
    Op = mybir.AluOpType
    Act = mybir.ActivationFunctionType
    ts = bass.ts

    a0 = [float(A[0, h]) for h in range(H)]
    a3 = [float(A[3, h]) for h in range(H)]
    skip_b1 = flags["skip_b1"]
    skip_ln = flags["skip_ln"]
    skip_b2 = flags["skip_b2"]
    general = not (skip_b1 and skip_ln and skip_b2)

    nc = bacc.Bacc("TRN2", target_bir_lowering=False, debug=False, num_devices=NCORES)

    rowflat_d = nc.dram_tensor("rowflat", [1, 6 * N], f32, kind="ExternalInput")
    colcat_d = nc.dram_tensor("colcat", [256, 6], f32, kind="ExternalInput")
    wv16_d = nc.dram_tensor("wv16", [16, E], f32, kind="ExternalInput")
    w1_d = nc.dram_tensor("w1", [E, E], f32, kind="ExternalInput")
    w2_d = nc.dram_tensor("w2", [E, E], f32, kind="ExternalInput")
    tailrow_d = nc.dram_tensor("tailrow", [1, 20], f32, kind="ExternalInput")
    if general:
        extrarow_d = nc.dram_tensor("extrarow", [1, 4 * E], f32, kind="ExternalInput")
    out_d = nc.dram_tensor("out", [256, E], f32, kind="ExternalOutput")

    def bcast(ap, parts):
        return bass.AP(tensor=ap.tensor, offset=ap.offset, ap=[[0, parts]] + list(ap.ap))

    with tile.TileContext(nc) as tc:
        with (
            tc.tile_pool(name="consts", bufs=1) as consts,
            tc.tile_pool(name="work", bufs=1) as work,
            tc.tile_pool(name="small", bufs=2) as small,
            tc.tile_pool(name="psum", bufs=1, space="PSUM") as psum,
        ):
            # ---- PX/PY broadcast via chunked DMAs on both HWDGE queues ----
            PX = consts.tile([128, N], f32)
            PY = consts.tile([128, N], f32)
            CH = 4
            for c in range(CH):
                sl = slice(c * (128 // CH), (c + 1) * (128 // CH))
                eng = nc.sync if c % 2 == 0 else nc.scalar
                eng.dma_start(out=PX[sl, :], in_=bcast(rowflat_d[0, 0:N], 128 // CH))
            for c in range(CH):
                sl = slice(c * (128 // CH), (c + 1) * (128 // CH))
                eng = nc.sync if c % 2 == 0 else nc.scalar
                eng.dma_start(out=PY[sl, :], in_=bcast(rowflat_d[0, N : 2 * N], 128 // CH))
            COLCAT = [consts.tile([128, 6], f32, name=f"COLCAT{it}") for it in range(2)]
            nc.sync.dma_start(out=COLCAT[0], in_=colcat_d[0:128, :])
            nc.scalar.dma_start(out=COLCAT[1], in_=colcat_d[128:256, :])
            ROWFLAT = consts.tile([1, 6 * N], f32)
            nc.sync.dma_start(out=ROWFLAT[:, 2 * N :], in_=rowflat_d[:, 2 * N :])
            TAILROW = consts.tile([1, 20], f32)
            nc.sync.dma_start(out=TAILROW, in_=tailrow_d[:, :])
            WV16 = consts.tile([16, E], f32)
            nc.scalar.dma_start(out=WV16, in_=wv16_d[:, :])
            W1S = consts.tile([E, E], f32)
            nc.scalar.dma_start(out=W1S, in_=w1_d[:, :])
            W2S = consts.tile([E, E], f32)
            nc.scalar.dma_start(out=W2S, in_=w2_d[:, :])
            if general:
                EXTRAROW = consts.tile([1, 4 * E], f32)
                nc.sync.dma_start(out=EXTRAROW, in_=extrarow_d[:, :])

            # ---- Act Sqrt table preload (dummy) + bias consts ----
            dum1 = consts.tile([128, 1], f32)
            nc.gpsimd.memset(dum1, 1.0)
            EPS_T = consts.tile([128, 1], f32)
            nc.gpsimd.memset(EPS_T, float(EPS))
            LNEPS_T = consts.tile([128, 1], f32)
            nc.gpsimd.memset(LNEPS_T, float(LN_EPS))
            CXB_T = consts.tile([128, 1], f32)
            nc.gpsimd.memset(CXB_T, float(CX_BIAS))
            dumo = consts.tile([128, 1], f32)
            nc.scalar.activation(dumo, dum1, Act.Sqrt)

            # ---- WR / tail consts broadcast on gpsimd ----
            WR = consts.tile([128, H, N], f32)
            for h in range(H):
                nc.gpsimd.partition_broadcast(
                    WR[:, h, :], ROWFLAT[0:1, (2 + h) * N : (3 + h) * N]
                )
            TAILC = consts.tile([128, 20], f32)
            nc.gpsimd.partition_broadcast(TAILC, TAILROW[0:1, :])
            ZC = TAILC[:, 0:4]
            SC = TAILC[:, 4:20]
            if general:
                EXTRAC = consts.tile([128, 4 * E], f32)
                nc.gpsimd.partition_broadcast(EXTRAC, EXTRAROW[0:1, :])
                B1R = EXTRAC[:, 0:E]
                GR = EXTRAC[:, E : 2 * E]
                BR = EXTRAC[:, 2 * E : 3 * E]
                B2R = EXTRAC[:, 3 * E : 4 * E]
            IDENT = consts.tile([128, 128], f32)
            make_identity(nc, IDENT)

            pcol0 = [COLCAT[it][:, 0:1] for it in range(2)]
            pcol1 = [COLCAT[it][:, 1:2] for it in range(2)]
            wcol = [[COLCAT[it][:, 2 + h : 3 + h] for h in range(H)] for it in range(2)]

            # ---- features: it-interleaved so Act stages batch ----
            def wtile(nm, it, dt=f32):
                return work.tile([128, N], dt, tag=f"{nm}{it}", name=f"{nm}{it}")

            dx, dy, dx2, dy2, r2, dist, inv, ld, cx, cy = ({} for _ in range(10))
            for it in range(2):
                dx[it] = wtile("dx", it)
                nc.vector.tensor_scalar_sub(dx[it], PX, pcol0[it])
                dy[it] = wtile("dy", it)
                nc.vector.tensor_scalar_sub(dy[it], PY, pcol1[it])
            for it in range(2):
                dx2[it] = wtile("dx2", it)
                nc.vector.tensor_mul(dx2[it], dx[it], dx[it])
                dy2[it] = wtile("dy2", it)
                nc.vector.tensor_mul(dy2[it], dy[it], dy[it])
            for it in range(2):
                r2[it] = wtile("r2", it)
                nc.vector.tensor_add(r2[it], dx2[it], dy2[it])
            for it in range(2):
                dist[it] = wtile("dist", it)
                nc.scalar.activation(dist[it], r2[it], Act.Sqrt, bias=EPS_T[:, :])
            for it in range(2):
                inv[it] = wtile("inv", it)
                nc.vector.reciprocal_approx_fast(out=inv[it], in_=dist[it])
            for it in range(2):
                cx[it] = wtile("cx", it)
                nc.vector.tensor_mul(cx[it], dx[it], inv[it])
                cy[it] = wtile("cy", it)
                nc.vector.tensor_mul(cy[it], dy[it], inv[it])
            # Ln block: ld (f32) + replay log-planes (bf16)
            lncx, lncy, lnld = {}, {}, {}
            for it in range(2):
                ld[it] = wtile("ld", it)
                nc.scalar.activation(ld[it], dist[it], Act.Ln, bias=1.0)
            if REPLAY_CXCY:
                for it in range(2):
                    lncx[it] = wtile("lncx", it, bf16)
                    nc.scalar.activation(lncx[it], cx[it], Act.Ln, bias=CXB_T[:, :])
                    lncy[it] = wtile("lncy", it, bf16)
                    nc.scalar.activation(lncy[it], cy[it], Act.Ln, bias=CXB_T[:, :])
            if REPLAY_LD:
                for it in range(2):
                    lnld[it] = wtile("lnld", it, bf16)
                    nc.scalar.activation(lnld[it], ld[it], Act.Ln, scale=LD_SCALE)

            # ---- logits + exp (E in bf16; l2 in bf16 for replay adds) ----
            Z, Es, l2b = {}, {}, {}
            S = {}
            junk = [
                work.tile([128, N], bf16, tag=f"junk{i}", name=f"junk{i}") for i in range(2)
            ]
            for it in range(2):
                Z[it] = small.tile([128, H], f32, tag=f"Z{it}", name=f"Z{it}")
                S[it] = small.tile([128, 16], f32, tag=f"S{it}", name=f"S{it}")
                Es[it] = []
                l2b[it] = []
                for h in range(H):
                    x = work.tile([128, N], f32, tag="x", name="x", bufs=2)
                    nc.vector.scalar_tensor_tensor(
                        x, WR[:, h, :], wcol[it][h], inv[it], op0=Op.subtract, op1=Op.mult
                    )
                    l1 = work.tile([128, N], f32, tag="l1", name="l1", bufs=2)
                    nc.vector.scalar_tensor_tensor(
                        l1, dist[it], a0[h], x, op0=Op.mult, op1=Op.add
                    )
                    l2 = work.tile([128, N], bf16, tag=f"l2_{h}_{it}", name=f"l2_{h}_{it}")
                    nc.vector.scalar_tensor_tensor(
                        l2, ld[it], a3[h], l1, op0=Op.mult, op1=Op.add
                    )
                    l2b[it].append(l2)
                    Eh = work.tile([128, N], bf16, tag=f"E{h}_{it}", name=f"E{h}_{it}")
                    nc.scalar.activation(Eh, l2, Act.Exp, accum_out=Z[it][:, h : h + 1])
                    Es[it].append(Eh)

            # ---- S-stage ----
            # dist products: DVE fused multiply-accumulate
            for it in range(2):
                for h in range(H):
                    prod = work.tile([128, N], bf16, tag="prod", name="prod", bufs=2)
                    nc.vector.scalar_tensor_tensor(
                        prod, Es[it][h], 1.0, dist[it], op0=Op.mult, op1=Op.mult,
                        accum_out=S[it][:, h * 4 : h * 4 + 1],
                    )

            # replay products: DVE bf16 add + Act exp-accumulate
            def replay(it, h, lnplane, col):
                addp = work.tile([128, N], bf16, tag="addp", name="addp", bufs=2)
                nc.vector.tensor_add(addp, l2b[it][h], lnplane)
                jt = junk[(h + it) % 2]
                nc.scalar.activation(jt, addp, Act.Exp, accum_out=S[it][:, col : col + 1])

            for it in range(2):
                for h in range(H):
                    if REPLAY_CXCY:
                        replay(it, h, lncx[it], h * 4 + 1)
                        replay(it, h, lncy[it], h * 4 + 2)
                    else:
                        for p, feat in ((1, cx[it]), (2, cy[it])):
                            prod = work.tile([128, N], bf16, tag="prod", name="prod", bufs=2)
                            nc.vector.scalar_tensor_tensor(
                                prod, Es[it][h], 1.0, feat, op0=Op.mult, op1=Op.mult,
                                accum_out=S[it][:, h * 4 + p : h * 4 + p + 1],
                            )
                    if REPLAY_LD:
                        replay(it, h, lnld[it], h * 4 + 3)
                    else:
                        prod = work.tile([128, N], bf16, tag="prod", name="prod", bufs=2)
                        nc.vector.scalar_tensor_tensor(
                            prod, Es[it][h], 1.0, ld[it], op0=Op.mult, op1=Op.mult,
                            accum_out=S[it][:, h * 4 + 3 : h * 4 + 4],
                        )

            # ---- per-tile tail ----
            for it in range(2):
                Zc = small.tile([128, H], f32, tag=f"Zc{it}", name=f"Zc{it}")
                nc.vector.tensor_sub(Zc, Z[it], ZC)
                Zi = small.tile([128, H], f32, tag=f"Zi{it}", name=f"Zi{it}")
                nc.vector.reciprocal(Zi, Zc)
                Sn = small.tile([128, 16], f32, tag=f"Sn{it}", name=f"Sn{it}")
                nc.vector.tensor_sub(Sn, S[it], SC)
                if REPLAY_CXCY:
                    # cx/cy columns: subtract runtime 2*Zc
                    for h in range(H):
                        for p in (1, 2):
                            c0 = h * 4 + p
                            nc.vector.scalar_tensor_tensor(
                                Sn[:, c0 : c0 + 1], Zc[:, h : h + 1], -2.0,
                                Sn[:, c0 : c0 + 1], op0=Op.mult, op1=Op.add,
                            )
                for h in range(H):
                    nc.vector.tensor_scalar_mul(
                        Sn[:, h * 4 : h * 4 + 4], Sn[:, h * 4 : h * 4 + 4], Zi[:, h : h + 1]
                    )
                ps_t = psum.tile([16, 128], f32, tag="ps_t", name="ps_t")
                nc.tensor.transpose(ps_t, Sn, IDENT)
                SNT = small.tile([16, 128], f32, tag=f"SNT{it}", name=f"SNT{it}")
                nc.vector.tensor_copy(SNT, ps_t)
                ctx_ps = psum.tile([128, 128], f32, tag="ctx_ps", name="ctx_ps")
                nc.tensor.matmul(ctx_ps, lhsT=WV16, rhs=SNT, start=True, stop=True)
                ctxT = small.tile([128, 128], f32, tag=f"ctxT{it}", name=f"ctxT{it}")
                nc.vector.tensor_copy(ctxT, ctx_ps)
                h1_ps = psum.tile([128, E], f32, tag="h1", name="h1")
                nc.tensor.matmul(h1_ps, lhsT=ctxT, rhs=W1S, start=True, stop=True)
                if skip_b1:
                    h1b = h1_ps
                else:
                    h1b = small.tile([128, E], f32, tag=f"h1b{it}", name=f"h1b{it}")
                    nc.vector.tensor_add(h1b, h1_ps, B1R)
                stats = small.tile([128, 6], f32, tag="stats", name="stats")
                nc.vector.bn_stats(stats, h1b)
                mv = small.tile([128, 2], f32, tag="mv", name="mv")
                nc.vector.bn_aggr(mv, stats)
                sd = small.tile([128, 1], f32, tag="sd", name="sd")
                nc.scalar.activation(sd, mv[:, 1:2], Act.Sqrt, bias=LNEPS_T[:, :])
                rstd = small.tile([128, 1], f32, tag="rstd", name="rstd")
                nc.vector.reciprocal(rstd, sd)
                xc = small.tile([128, E], f32, tag="xc", name="xc")
                nc.vector.tensor_scalar(
                    xc, h1b, scalar1=mv[:, 0:1], scalar2=rstd, op0=Op.subtract, op1=Op.mult
                )
                y2 = xc
                if not skip_ln:
                    y1 = small.tile([128, E], f32, tag="y1", name="y1")
                    nc.vector.tensor_mul(y1, xc, GR)
                    y2 = small.tile([128, E], f32, tag="y2", name="y2")
                    nc.vector.tensor_add(y2, y1, BR)
                g = small.tile([128, E], f32, tag="g", name="g")
                nc.scalar.activation(g, y2, Act.Gelu)
                g_ps = psum.tile([128, 128], f32, tag="g_ps", name="g_ps")
                nc.tensor.transpose(g_ps, g, IDENT)
                gT = small.tile([128, 128], f32, tag="gT", name="gT")
                nc.vector.tensor_copy(gT, g_ps)
                h2_ps = psum.tile([128, E], f32, tag="h2", name="h2")
                nc.tensor.matmul(h2_ps, lhsT=gT, rhs=W2S, start=True, stop=True)
                outt = small.tile([128, E], f32, tag=f"outt{it}", name=f"outt{it}")
                if skip_b2:
                    nc.vector.tensor_copy(outt, h2_ps)
                else:
                    nc.vector.tensor_add(outt, h2_ps, B2R)
                eng = nc.sync if it == 0 else nc.scalar
                eng.dma_start(out=out_d[ts(it, 128), :], in_=outt)

    nc.compile()
    return nc


last_results = None


def kernel(positions, key_padding_mask, kv_w, kv_b, query, w1, b1, ln_g, ln_b, w2, b2):
    from concourse.bass_utils import run_bass_kernel_spmd

    per_core, A, flags = _host_prep(
        positions, key_padding_mask, kv_w, kv_b, query, w1, b1, ln_g, ln_b, w2, b2
    )
    nc = _build_program(A, flags)
    res = run_bass_kernel_spmd(nc, per_core, core_ids=list(range(NCORES)))
    global last_results
    last_results = res
    out = np.empty((B, N, E), dtype=np.float32)
    for c in range(NCORES):
        b = c // 2
        i0 = (c % 2) * 256
        out[b, i0 : i0 + 256] = res.results[c]["out"]
    return out


# revision 12
# speedup vs baseline: 1.0925x; 1.0925x over previous
"""Trainium2 Bass kernel for NeighborhoodAggregationEmbedding.

Math (reference):
  rel features per pair (i,j): dist, cos, sin, dx/(dist+eps), dy/(dist+eps), log1p(dist)
  kv = feats @ kv_w + kv_b ; k,v heads ; logits = q.k/sqrt(D); softmax over j
  (self-masked, pad-masked); ctx = attn.v ; MLP: LN(ctx@w1+b1) -> gelu -> @w2+b2

Key algebraic restructure (host-side, exact up to ~1e-7):
  * cos ~= dx/dist, sin ~= dy/dist so the 6 features collapse to 4:
    F = [dist, cx, cy, log1p(dist)].
  * query is shared by every (b, i) so logits = F @ A with a host-computed
    (4,4) matrix A; the cx/cy logit terms become (w[j]-w[i])*inv with
    w = a1*px + a2*py per node (padding folds into w[j] as -1e20).
  * attn.v  ==>  S[i,h,p] = sum_j E_h * F_p ; ctx = (S/Z) @ Wv16.
  * self-mask via analytic diagonal corrections on Z and S.
  * |logits| < ~1 for this input distribution (A ~ 1e-3), so bf16
    intermediates after the logit are safe.

Device strategy (v4):
  * "exp-replay": for F in {cx,cy,ld}, sum_j E*F = sum_j exp(l2 + ln F')
    computed on the (otherwise idle) scalar engine: DVE does one cheap
    bf16 2x add (l2b + lnF'b), Act does exp with accumulate. F' is
    range-shifted/scaled so ln F' is small where terms matter
    (cx+2, ld/4); scales fold into Wv16 / diag corrections host-side.
    Only the dist products stay as DVE fused multiply-accumulates.
  * inv = reciprocal_approx_fast (5x faster than exact reciprocal).
  * PX/PY broadcast to 128 partitions via chunked HBM DMAs split across
    the two HWDGE queues; WR/tail constants via gpsimd partition_broadcast
    (gpsimd never runs concurrently with DVE compute - SBUF contention
    halves DVE throughput).
  * activation-table switches minimized (Sqrt preload; Ln/Exp blocks).
  * gelu via exact-erf Gelu activation; LN gamma/beta and biases skipped
    on device when the host detects identity/zero values.

Per-core work (8 cores): core c -> batch b=c//2, query rows i in
[256*(c%2), 256*(c%2)+256); two [128 i x 512 j] tiles.
"""

import numpy as np

B, N, E, H = 4, 512, 128, 4
D = E // H
EPS = 1e-8
LN_EPS = 1e-5
BIG = 1e20
NCORES = 8

_f32 = np.float32

LD_SCALE = 0.25          # replay plane: ln(ld * LD_SCALE)
CX_BIAS = 2.0            # replay plane: ln(cx + 2)
REPLAY_LD = True
REPLAY_CXCY = True


def _host_prep(positions, key_padding_mask, kv_w, kv_b, query, w1, b1, ln_g, ln_b, w2, b2):
    pos = np.asarray(positions, dtype=_f32)
    pad = np.asarray(key_padding_mask).astype(bool)
    kv_w = np.asarray(kv_w, dtype=_f32)
    kv_b = np.asarray(kv_b, dtype=_f32)
    q = np.asarray(query, dtype=_f32).reshape(H, D)
    w1 = np.asarray(w1, dtype=_f32)
    b1 = np.asarray(b1, dtype=_f32)
    ln_g = np.asarray(ln_g, dtype=_f32)
    ln_b = np.asarray(ln_b, dtype=_f32)
    w2 = np.asarray(w2, dtype=_f32)
    b2 = np.asarray(b2, dtype=_f32)

    Wk = kv_w[:, :E]
    Wv = kv_w[:, E:]
    Wk4 = np.stack([Wk[0], Wk[1] + Wk[3], Wk[2] + Wk[4], Wk[5]]).astype(_f32)
    Wv4 = np.stack([Wv[0], Wv[1] + Wv[3], Wv[2] + Wv[4], Wv[5]]).astype(_f32)

    A = np.einsum("phd,hd->ph", Wk4.reshape(4, H, D), q) / np.sqrt(_f32(D))
    A = A.astype(_f32)

    b1_eff = (b1 + kv_b[E:] @ w1).astype(_f32)
    skip_b1 = bool(np.all(np.abs(b1_eff) < 1e-12))
    skip_ln = bool(np.all(ln_g == 1.0) and np.all(ln_b == 0.0))
    skip_b2 = bool(np.all(b2 == 0.0))

    wrow_nopad = (
        A[1][None, :, None] * pos[:, None, :, 0] + A[2][None, :, None] * pos[:, None, :, 1]
    ).astype(_f32)
    wrow = (wrow_nopad - _f32(BIG) * pad[:, None, :].astype(_f32)).astype(_f32)

    # analytic device diagonal values
    d0 = _f32(np.sqrt(_f32(EPS)))
    ld0 = _f32(np.log(_f32(1.0) + d0))
    e_diag = np.exp((A[0] * d0 + A[3] * ld0).astype(_f32)).astype(_f32)
    zcorr = e_diag.copy()
    # scorr is in ACCUMULATOR units per column (replay columns accumulate
    # scaled quantities). For replayed cx/cy columns S = R - 2*Z_all =
    # R - 2*Zc - 2*zcorr; the -2*Zc part is runtime, 2*zcorr is static here.
    scorr = np.zeros(16, dtype=_f32)
    for h in range(H):
        scorr[h * 4 + 0] = e_diag[h] * d0
        if REPLAY_CXCY:
            scorr[h * 4 + 1] = 2.0 * zcorr[h]
            scorr[h * 4 + 2] = 2.0 * zcorr[h]
        if REPLAY_LD:
            scorr[h * 4 + 3] = e_diag[h] * ld0 * LD_SCALE
        else:
            scorr[h * 4 + 3] = e_diag[h] * ld0
    scorr = scorr.astype(_f32)

    # Wv16[(h,p), e] = Wv4[p, e] restricted to head-h block; replayed ld
    # columns accumulate E*ld*LD_SCALE so those rows get rescaled.
    Wv16 = np.zeros((16, E), dtype=_f32)
    for h in range(H):
        for p in range(4):
            r = Wv4[p, h * D : (h + 1) * D]
            if p == 3 and REPLAY_LD:
                r = r / _f32(LD_SCALE)
            Wv16[h * 4 + p, h * D : (h + 1) * D] = r

    tailrow = np.concatenate([zcorr, scorr])[None, :].astype(_f32)  # [1, 20]

    shared = {
        "wv16": Wv16.astype(_f32),
        "w1": w1,
        "w2": w2,
        "tailrow": tailrow,
    }
    per_core = []
    for c in range(NCORES):
        b = c // 2
        i0 = (c % 2) * 256
        rowflat = np.concatenate([pos[b, :, 0], pos[b, :, 1], wrow[b].reshape(-1)])[None, :]
        colcat = np.concatenate(
            [pos[b, i0 : i0 + 256], wrow_nopad[b, :, i0 : i0 + 256].T], axis=1
        )
        per_core.append(
            {
                "rowflat": np.ascontiguousarray(rowflat, dtype=_f32),
                "colcat": np.ascontiguousarray(colcat, dtype=_f32),
                **shared,
            }
        )
    flags = {"skip_b1": skip_b1, "skip_ln": skip_ln, "skip_b2": skip_b2}
    if not (skip_b1 and skip_ln and skip_b2):
        extra = np.concatenate([b1_eff, ln_g, ln_b, b2])[None, :].astype(_f32)
        for pc in per_core:
            pc["extrarow"] = extra
    return per_core, A, flags


def _build_program(A, flags):
    import concourse.bacc as bacc
    import concourse.bass as bass
    import concourse.tile as tile
    from concourse import mybir
    from concourse.masks import make_identity

    f32 = mybir.dt.float32
    bf16 = mybir.dt.bfloat16
    Op = mybir.AluOpType
    Act = mybir.ActivationFunctionType
    ts = bass.ts

    a0 = [float(A[0, h]) for h in range(H)]
    a3 = [float(A[3, h]) for h in range(H)]
    skip_b1 = flags["skip_b1"]
    skip_ln = flags["skip_ln"]
    skip_b2 = flags["skip_b2"]
    general = not (skip_b1 and skip_ln and skip_b2)

    nc = bacc.Bacc("TRN2", target_bir_lowering=False, debug=False, num_devices=NCORES)

    rowflat_d = nc.dram_tensor("rowflat", [1, 6 * N], f32, kind="ExternalInput")
    colcat_d = nc.dram_tensor("colcat", [256, 6], f32, kind="ExternalInput")
    wv16_d = nc.dram_tensor("wv16", [16, E], f32, kind="ExternalInput")
    w1_d = nc.dram_tensor("w1", [E, E], f32, kind="ExternalInput")
    w2_d = nc.dram_tensor("w2", [E, E], f32, kind="ExternalInput")
    tailrow_d = nc.dram_tensor("tailrow", [1, 20], f32, kind="ExternalInput")
    if general:
        extrarow_d = nc.dram_tensor("extrarow", [1, 4 * E], f32, kind="ExternalInput")
    out_d = nc.dram_tensor("out", [256, E], f32, kind="ExternalOutput")

    def bcast(ap, parts):
        return bass.AP(tensor=ap.tensor, offset=ap.offset, ap=[[0, parts]] + list(ap.ap))

    with tile.TileContext(nc) as tc:
        with (
            tc.tile_pool(name="consts", bufs=1) as consts,
            tc.tile_pool(name="work", bufs=1) as work,
            tc.tile_pool(name="small", bufs=2) as small,
            tc.tile_pool(name="psum", bufs=1, space="PSUM") as psum,
        ):
            # ---- PX/PY broadcast via chunked DMAs on both HWDGE queues ----
            PX = consts.tile([128, N], f32)
            PY = consts.tile([128, N], f32)
            CH = 4
            for c in range(CH):
                sl = slice(c * (128 // CH), (c + 1) * (128 // CH))
                eng = nc.sync if c % 2 == 0 else nc.scalar
                eng.dma_start(out=PX[sl, :], in_=bcast(rowflat_d[0, 0:N], 128 // CH))
            for c in range(CH):
                sl = slice(c * (128 // CH), (c + 1) * (128 // CH))
                eng = nc.sync if c % 2 == 0 else nc.scalar
                eng.dma_start(out=PY[sl, :], in_=bcast(rowflat_d[0, N : 2 * N], 128 // CH))
            COLCAT = [consts.tile([128, 6], f32, name=f"COLCAT{it}") for it in range(2)]
            nc.sync.dma_start(out=COLCAT[0], in_=colcat_d[0:128, :])
            nc.scalar.dma_start(out=COLCAT[1], in_=colcat_d[128:256, :])
            ROWFLAT = consts.tile([1, 6 * N], f32)
            nc.sync.dma_start(out=ROWFLAT[:, 2 * N :], in_=rowflat_d[:, 2 * N :])
            TAILROW = consts.tile([1, 20], f32)
            nc.sync.dma_start(out=TAILROW, in_=tailrow_d[:, :])
            WV16 = consts.tile([16, E], f32)
            nc.scalar.dma_start(out=WV16, in_=wv16_d[:, :])
            W1S = consts.tile([E, E], f32)
            nc.scalar.dma_start(out=W1S, in_=w1_d[:, :])
            W2S = consts.tile([E, E], f32)
            nc.scalar.dma_start(out=W2S, in_=w2_d[:, :])
            if general:
                EXTRAROW = consts.tile([1, 4 * E], f32)
                nc.sync.dma_start(out=EXTRAROW, in_=extrarow_d[:, :])

            # ---- Act Sqrt table preload (dummy) + bias consts ----
            dum1 = consts.tile([128, 1], f32)
            nc.gpsimd.memset(dum1, 1.0)
            EPS_T = consts.tile([128, 1], f32)
            nc.gpsimd.memset(EPS_T, float(EPS))
            LNEPS_T = consts.tile([128, 1], f32)
            nc.gpsimd.memset(LNEPS_T, float(LN_EPS))
            CXB_T = consts.tile([128, 1], f32)
            nc.gpsimd.memset(CXB_T, float(CX_BIAS))
            dumo = consts.tile([128, 1], f32)
            nc.scalar.activation(dumo, dum1, Act.Sqrt)

            # ---- WR / tail consts broadcast on gpsimd ----
            WR = consts.tile([128, H, N], f32)
            for h in range(H):
                nc.gpsimd.partition_broadcast(
                    WR[:, h, :], ROWFLAT[0:1, (2 + h) * N : (3 + h) * N]
                )
            TAILC = consts.tile([128, 20], f32)
            nc.gpsimd.partition_broadcast(TAILC, TAILROW[0:1, :])
            ZC = TAILC[:, 0:4]
            SC = TAILC[:, 4:20]
            if general:
                EXTRAC = consts.tile([128, 4 * E], f32)
                nc.gpsimd.partition_broadcast(EXTRAC, EXTRAROW[0:1, :])
                B1R = EXTRAC[:, 0:E]
                GR = EXTRAC[:, E : 2 * E]
                BR = EXTRAC[:, 2 * E : 3 * E]
                B2R = EXTRAC[:, 3 * E : 4 * E]
            IDENT = consts.tile([128, 128], f32)
            make_identity(nc, IDENT)

            pcol0 = [COLCAT[it][:, 0:1] for it in range(2)]
            pcol1 = [COLCAT[it][:, 1:2] for it in range(2)]
            wcol = [[COLCAT[it][:, 2 + h : 3 + h] for h in range(H)] for it in range(2)]

            # ---- features: it-interleaved so Act stages batch ----
            def wtile(nm, it, dt=f32):
                return work.tile([128, N], dt, tag=f"{nm}{it}", name=f"{nm}{it}")

            dx, dy, dx2, dy2, r2, dist, inv, ld, cx, cy = ({} for _ in range(10))
            for it in range(2):
                dx[it] = wtile("dx", it)
                nc.vector.tensor_scalar_sub(dx[it], PX, pcol0[it])
                dy[it] = wtile("dy", it)
                nc.vector.tensor_scalar_sub(dy[it], PY, pcol1[it])
            for it in range(2):
                dx2[it] = wtile("dx2", it)
                nc.vector.tensor_mul(dx2[it], dx[it], dx[it])
                dy2[it] = wtile("dy2", it)
                nc.vector.tensor_mul(dy2[it], dy[it], dy[it])
            for it in range(2):
                r2[it] = wtile("r2", it)
                nc.vector.tensor_add(r2[it], dx2[it], dy2[it])
            for it in range(2):
                dist[it] = wtile("dist", it)
                nc.scalar.activation(dist[it], r2[it], Act.Sqrt, bias=EPS_T[:, :])
            for it in range(2):
                inv[it] = wtile("inv", it)
                nc.vector.reciprocal_approx_fast(out=inv[it], in_=dist[it])
            for it in range(2):
                cx[it] = wtile("cx", it)
                nc.vector.tensor_mul(cx[it], dx[it], inv[it])
                cy[it] = wtile("cy", it)
                nc.vector.tensor_mul(cy[it], dy[it], inv[it])
            # Ln block: ld (f32) + replay log-planes (bf16)
            lncx, lncy, lnld = {}, {}, {}
            for it in range(2):
                ld[it] = wtile("ld", it)
                nc.scalar.activation(ld[it], dist[it], Act.Ln, bias=1.0)
            if REPLAY_CXCY:
                for it in range(2):
                    lncx[it] = wtile("lncx", it, bf16)
                    nc.scalar.activation(lncx[it], cx[it], Act.Ln, bias=CXB_T[:, :])
                    lncy[it] = wtile("lncy", it, bf16)
                    nc.scalar.activation(lncy[it], cy[it], Act.Ln, bias=CXB_T[:, :])
            if REPLAY_LD:
                for it in range(2):
                    lnld[it] = wtile("lnld", it, bf16)
                    nc.scalar.activation(lnld[it], ld[it], Act.Ln, scale=LD_SCALE)

            # ---- logits + exp (E in bf16; l2 in bf16 for replay adds) ----
            Z, Es, l2b = {}, {}, {}
            junk = [
                work.tile([128, N], bf16, tag=f"junk{i}", name=f"junk{i}") for i in range(2)
            ]
            Sd, Sa = {}, {}
            for it in range(2):
                Z[it] = small.tile([128, H], f32, tag=f"Z{it}", name=f"Z{it}")
                Sd[it] = small.tile([128, 16], f32, tag=f"Sd{it}", name=f"Sd{it}")
                nc.gpsimd.memset(Sd[it], 0.0)
                Sa[it] = small.tile([128, 16], f32, tag=f"Sa{it}", name=f"Sa{it}")
                nc.gpsimd.memset(Sa[it], 0.0)
                Es[it] = []
                l2b[it] = []
                for h in range(H):
                    x = work.tile([128, N], f32, tag="x", name="x", bufs=2)
                    nc.vector.scalar_tensor_tensor(
                        x, WR[:, h, :], wcol[it][h], inv[it], op0=Op.subtract, op1=Op.mult
                    )
                    l1 = work.tile([128, N], f32, tag="l1", name="l1", bufs=2)
                    nc.vector.scalar_tensor_tensor(
                        l1, dist[it], a0[h], x, op0=Op.mult, op1=Op.add
                    )
                    l2 = work.tile([128, N], bf16, tag=f"l2_{h}_{it}", name=f"l2_{h}_{it}")
                    nc.vector.scalar_tensor_tensor(
                        l2, ld[it], a3[h], l1, op0=Op.mult, op1=Op.add
                    )
                    l2b[it].append(l2)
                    Eh = work.tile([128, N], bf16, tag=f"E{h}_{it}", name=f"E{h}_{it}")
                    nc.scalar.activation(Eh, l2, Act.Exp, accum_out=Z[it][:, h : h + 1])
                    Es[it].append(Eh)

            # ---- S-stage ----
            # dist products: DVE fused multiply-accumulate
            for it in range(2):
                for h in range(H):
                    prod = work.tile([128, N], bf16, tag="prod", name="prod", bufs=2)
                    nc.vector.scalar_tensor_tensor(
                        prod, Es[it][h], 1.0, dist[it], op0=Op.mult, op1=Op.mult,
                        accum_out=Sd[it][:, h * 4 : h * 4 + 1],
                    )

            # replay products: DVE bf16 add + Act exp-accumulate
            def replay(it, h, lnplane, col):
                addp = work.tile([128, N], bf16, tag="addp", name="addp", bufs=4)
                nc.vector.tensor_add(addp, l2b[it][h], lnplane)
                jt = junk[(h + it) % 2]
                nc.scalar.activation(jt, addp, Act.Exp, accum_out=Sa[it][:, col : col + 1])

            for it in range(2):
                for h in range(H):
                    if REPLAY_CXCY:
                        replay(it, h, lncx[it], h * 4 + 1)
                        replay(it, h, lncy[it], h * 4 + 2)
                    else:
                        for p, feat in ((1, cx[it]), (2, cy[it])):
                            prod = work.tile([128, N], bf16, tag="prod", name="prod", bufs=2)
                            nc.vector.scalar_tensor_tensor(
                                prod, Es[it][h], 1.0, feat, op0=Op.mult, op1=Op.mult,
                                accum_out=Sd[it][:, h * 4 + p : h * 4 + p + 1],
                            )
                    if REPLAY_LD:
                        replay(it, h, lnld[it], h * 4 + 3)
                    else:
                        prod = work.tile([128, N], bf16, tag="prod", name="prod", bufs=2)
                        nc.vector.scalar_tensor_tensor(
                            prod, Es[it][h], 1.0, ld[it], op0=Op.mult, op1=Op.mult,
                            accum_out=Sd[it][:, h * 4 + 3 : h * 4 + 4],
                        )

            # ---- per-tile tail ----
            for it in range(2):
                Zc = small.tile([128, H], f32, tag=f"Zc{it}", name=f"Zc{it}")
                nc.vector.tensor_sub(Zc, Z[it], ZC)
                Zi = small.tile([128, H], f32, tag=f"Zi{it}", name=f"Zi{it}")
                nc.vector.reciprocal(Zi, Zc)
                Ssum = small.tile([128, 16], f32, tag=f"Ssum{it}", name=f"Ssum{it}")
                nc.vector.tensor_add(Ssum, Sd[it], Sa[it])
                Sn = small.tile([128, 16], f32, tag=f"Sn{it}", name=f"Sn{it}")
                nc.vector.tensor_sub(Sn, Ssum, SC)
                if REPLAY_CXCY:
                    # cx/cy columns: subtract runtime 2*Zc
                    for h in range(H):
                        for p in (1, 2):
                            c0 = h * 4 + p
                            nc.vector.scalar_tensor_tensor(
                                Sn[:, c0 : c0 + 1], Zc[:, h : h + 1], -2.0,
                                Sn[:, c0 : c0 + 1], op0=Op.mult, op1=Op.add,
                            )
                for h in range(H):
                    nc.vector.tensor_scalar_mul(
                        Sn[:, h * 4 : h * 4 + 4], Sn[:, h * 4 : h * 4 + 4], Zi[:, h : h + 1]
                    )
                ps_t = psum.tile([16, 128], f32, tag="ps_t", name="ps_t")
                nc.tensor.transpose(ps_t, Sn, IDENT)
                SNT = small.tile([16, 128], f32, tag=f"SNT{it}", name=f"SNT{it}")
                nc.vector.tensor_copy(SNT, ps_t)
                ctx_ps = psum.tile([128, 128], f32, tag="ctx_ps", name="ctx_ps")
                nc.tensor.matmul(ctx_ps, lhsT=WV16, rhs=SNT, start=True, stop=True)
                ctxT = small.tile([128, 128], f32, tag=f"ctxT{it}", name=f"ctxT{it}")
                nc.vector.tensor_copy(ctxT, ctx_ps)
                h1_ps = psum.tile([128, E], f32, tag="h1", name="h1")
                nc.tensor.matmul(h1_ps, lhsT=ctxT, rhs=W1S, start=True, stop=True)
                if skip_b1:
                    h1b = h1_ps
                else:
                    h1b = small.tile([128, E], f32, tag=f"h1b{it}", name=f"h1b{it}")
                    nc.vector.tensor_add(h1b, h1_ps, B1R)
                stats = small.tile([128, 6], f32, tag="stats", name="stats")
                nc.vector.bn_stats(stats, h1b)
                mv = small.tile([128, 2], f32, tag="mv", name="mv")
                nc.vector.bn_aggr(mv, stats)
                sd = small.tile([128, 1], f32, tag="sd", name="sd")
                nc.scalar.activation(sd, mv[:, 1:2], Act.Sqrt, bias=LNEPS_T[:, :])
                rstd = small.tile([128, 1], f32, tag="rstd", name="rstd")
                nc.vector.reciprocal(rstd, sd)
                xc = small.tile([128, E], f32, tag="xc", name="xc")
                nc.vector.tensor_scalar(
                    xc, h1b, scalar1=mv[:, 0:1], scalar2=rstd, op0=Op.subtract, op1=Op.mult
                )
                y2 = xc
                if not skip_ln:
                    y1 = small.tile([128, E], f32, tag="y1", name="y1")
                    nc.vector.tensor_mul(y1, xc, GR)
                    y2 = small.tile([128, E], f32, tag="y2", name="y2")
                    nc.vector.tensor_add(y2, y1, BR)
                g = small.tile([128, E], f32, tag="g", name="g")
                nc.scalar.activation(g, y2, Act.Gelu)
                g_ps = psum.tile([128, 128], f32, tag="g_ps", name="g_ps")
                nc.tensor.transpose(g_ps, g, IDENT)
                gT = small.tile([128, 128], f32, tag="gT", name="gT")
                nc.vector.tensor_copy(gT, g_ps)
                h2_ps = psum.tile([128, E], f32, tag="h2", name="h2")
                nc.tensor.matmul(h2_ps, lhsT=gT, rhs=W2S, start=True, stop=True)
                outt = small.tile([128, E], f32, tag=f"outt{it}", name=f"outt{it}")
                if skip_b2:
                    nc.vector.tensor_copy(outt, h2_ps)
                else:
                    nc.vector.tensor_add(outt, h2_ps, B2R)
                eng = nc.sync if it == 0 else nc.scalar
                eng.dma_start(out=out_d[ts(it, 128), :], in_=outt)

    nc.compile()
    return nc


last_results = None


def kernel(positions, key_padding_mask, kv_w, kv_b, query, w1, b1, ln_g, ln_b, w2, b2):
    from concourse.bass_utils import run_bass_kernel_spmd

    per_core, A, flags = _host_prep(
        positions, key_padding_mask, kv_w, kv_b, query, w1, b1, ln_g, ln_b, w2, b2
    )
    nc = _build_program(A, flags)
    res = run_bass_kernel_spmd(nc, per_core, core_ids=list(range(NCORES)))
    global last_results
    last_results = res
    out = np.empty((B, N, E), dtype=np.float32)
    for c in range(NCORES):
        b = c // 2
        i0 = (c % 2) * 256
        out[b, i0 : i0 + 256] = res.results[c]["out"]
    return out


# revision 13
# speedup vs baseline: 1.2052x; 1.1031x over previous
"""Trainium2 Bass kernel for NeighborhoodAggregationEmbedding.

Math (reference):
  rel features per pair (i,j): dist, cos, sin, dx/(dist+eps), dy/(dist+eps), log1p(dist)
  kv = feats @ kv_w + kv_b ; k,v heads ; logits = q.k/sqrt(D); softmax over j
  (self-masked, pad-masked); ctx = attn.v ; MLP: LN(ctx@w1+b1) -> gelu -> @w2+b2

Key algebraic restructure (host-side, exact up to ~1e-7):
  * cos ~= dx/dist, sin ~= dy/dist so the 6 features collapse to 4:
    F = [dist, cx, cy, log1p(dist)].
  * query is shared by every (b, i) so logits = F @ A with a host-computed
    (4,4) matrix A; the cx/cy logit terms become (w[j]-w[i])*inv with
    w = a1*px + a2*py per node (padding folds into w[j] as -1e20).
  * attn.v  ==>  S[i,h,p] = sum_j E_h * F_p ; ctx = (S/Z) @ Wv16.
  * self-mask via analytic diagonal corrections on Z and S.
  * |logits| < ~1 for this input distribution (A ~ 1e-3), so bf16
    intermediates after the logit are safe.

Device strategy (v4):
  * "exp-replay": for F in {cx,cy,ld}, sum_j E*F = sum_j exp(l2 + ln F')
    computed on the (otherwise idle) scalar engine: DVE does one cheap
    bf16 2x add (l2b + lnF'b), Act does exp with accumulate. F' is
    range-shifted/scaled so ln F' is small where terms matter
    (cx+2, ld/4); scales fold into Wv16 / diag corrections host-side.
    Only the dist products stay as DVE fused multiply-accumulates.
  * inv = reciprocal_approx_fast (5x faster than exact reciprocal).
  * PX/PY broadcast to 128 partitions via chunked HBM DMAs split across
    the two HWDGE queues; WR/tail constants via gpsimd partition_broadcast
    (gpsimd never runs concurrently with DVE compute - SBUF contention
    halves DVE throughput).
  * activation-table switches minimized (Sqrt preload; Ln/Exp blocks).
  * gelu via exact-erf Gelu activation; LN gamma/beta and biases skipped
    on device when the host detects identity/zero values.

Per-core work (8 cores): core c -> batch b=c//2, query rows i in
[256*(c%2), 256*(c%2)+256); two [128 i x 512 j] tiles.
"""

import numpy as np

B, N, E, H = 4, 512, 128, 4
D = E // H
EPS = 1e-8
LN_EPS = 1e-5
BIG = 1e20
NCORES = 8

_f32 = np.float32

LD_SCALE = 0.25          # replay plane: ln(ld * LD_SCALE)
CX_BIAS = 2.0            # replay plane: ln(cx + 2)
REPLAY_LD = True
REPLAY_CXCY = True


def _host_prep(positions, key_padding_mask, kv_w, kv_b, query, w1, b1, ln_g, ln_b, w2, b2):
    pos = np.asarray(positions, dtype=_f32)
    pad = np.asarray(key_padding_mask).astype(bool)
    kv_w = np.asarray(kv_w, dtype=_f32)
    kv_b = np.asarray(kv_b, dtype=_f32)
    q = np.asarray(query, dtype=_f32).reshape(H, D)
    w1 = np.asarray(w1, dtype=_f32)
    b1 = np.asarray(b1, dtype=_f32)
    ln_g = np.asarray(ln_g, dtype=_f32)
    ln_b = np.asarray(ln_b, dtype=_f32)
    w2 = np.asarray(w2, dtype=_f32)
    b2 = np.asarray(b2, dtype=_f32)

    Wk = kv_w[:, :E]
    Wv = kv_w[:, E:]
    Wk4 = np.stack([Wk[0], Wk[1] + Wk[3], Wk[2] + Wk[4], Wk[5]]).astype(_f32)
    Wv4 = np.stack([Wv[0], Wv[1] + Wv[3], Wv[2] + Wv[4], Wv[5]]).astype(_f32)

    A = np.einsum("phd,hd->ph", Wk4.reshape(4, H, D), q) / np.sqrt(_f32(D))
    A = A.astype(_f32)

    b1_eff = (b1 + kv_b[E:] @ w1).astype(_f32)
    skip_b1 = bool(np.all(np.abs(b1_eff) < 1e-12))
    skip_ln = bool(np.all(ln_g == 1.0) and np.all(ln_b == 0.0))
    skip_b2 = bool(np.all(b2 == 0.0))

    wrow_nopad = (
        A[1][None, :, None] * pos[:, None, :, 0] + A[2][None, :, None] * pos[:, None, :, 1]
    ).astype(_f32)
    wrow = (wrow_nopad - _f32(BIG) * pad[:, None, :].astype(_f32)).astype(_f32)

    # analytic device diagonal values
    d0 = _f32(np.sqrt(_f32(EPS)))
    ld0 = _f32(np.log(_f32(1.0) + d0))
    e_diag = np.exp((A[0] * d0 + A[3] * ld0).astype(_f32)).astype(_f32)
    zcorr = e_diag.copy()
    # scorr is in ACCUMULATOR units per column (replay columns accumulate
    # scaled quantities). For replayed cx/cy columns S = R - 2*Z_all =
    # R - 2*Zc - 2*zcorr; the -2*Zc part is runtime, 2*zcorr is static here.
    scorr = np.zeros(16, dtype=_f32)
    for h in range(H):
        scorr[h * 4 + 0] = e_diag[h] * d0
        if REPLAY_CXCY:
            scorr[h * 4 + 1] = 2.0 * zcorr[h]
            scorr[h * 4 + 2] = 2.0 * zcorr[h]
        if REPLAY_LD:
            scorr[h * 4 + 3] = e_diag[h] * ld0 * LD_SCALE
        else:
            scorr[h * 4 + 3] = e_diag[h] * ld0
    scorr = scorr.astype(_f32)

    # Wv16[(h,p), e] = Wv4[p, e] restricted to head-h block; replayed ld
    # columns accumulate E*ld*LD_SCALE so those rows get rescaled.
    Wv16 = np.zeros((16, E), dtype=_f32)
    for h in range(H):
        for p in range(4):
            r = Wv4[p, h * D : (h + 1) * D]
            if p == 3 and REPLAY_LD:
                r = r / _f32(LD_SCALE)
            Wv16[h * 4 + p, h * D : (h + 1) * D] = r

    tailrow = np.concatenate([zcorr, scorr])[None, :].astype(_f32)  # [1, 20]

    shared = {
        "wv16": Wv16.astype(_f32),
        "w1": w1,
        "w2": w2,
        "tailrow": tailrow,
    }
    per_core = []
    for c in range(NCORES):
        b = c // 2
        i0 = (c % 2) * 256
        rowflat = np.concatenate([pos[b, :, 0], pos[b, :, 1], wrow[b].reshape(-1)])[None, :]
        colcat = np.concatenate(
            [pos[b, i0 : i0 + 256], wrow_nopad[b, :, i0 : i0 + 256].T], axis=1
        )
        per_core.append(
            {
                "rowflat": np.ascontiguousarray(rowflat, dtype=_f32),
                "colcat": np.ascontiguousarray(colcat, dtype=_f32),
                **shared,
            }
        )
    flags = {"skip_b1": skip_b1, "skip_ln": skip_ln, "skip_b2": skip_b2}
    if not (skip_b1 and skip_ln and skip_b2):
        extra = np.concatenate([b1_eff, ln_g, ln_b, b2])[None, :].astype(_f32)
        for pc in per_core:
            pc["extrarow"] = extra
    return per_core, A, flags


def _build_program(A, flags):
    import concourse.bacc as bacc
    import concourse.bass as bass
    import concourse.tile as tile
    from concourse import mybir
    from concourse.masks import make_identity

    f32 = mybir.dt.float32
    bf16 = mybir.dt.bfloat16
    Op = mybir.AluOpType
    Act = mybir.ActivationFunctionType
    ts = bass.ts

    a0 = [float(A[0, h]) for h in range(H)]
    a3 = [float(A[3, h]) for h in range(H)]
    skip_b1 = flags["skip_b1"]
    skip_ln = flags["skip_ln"]
    skip_b2 = flags["skip_b2"]
    general = not (skip_b1 and skip_ln and skip_b2)

    nc = bacc.Bacc("TRN2", target_bir_lowering=False, debug=False, num_devices=NCORES)

    rowflat_d = nc.dram_tensor("rowflat", [1, 6 * N], f32, kind="ExternalInput")
    colcat_d = nc.dram_tensor("colcat", [256, 6], f32, kind="ExternalInput")
    wv16_d = nc.dram_tensor("wv16", [16, E], f32, kind="ExternalInput")
    w1_d = nc.dram_tensor("w1", [E, E], f32, kind="ExternalInput")
    w2_d = nc.dram_tensor("w2", [E, E], f32, kind="ExternalInput")
    tailrow_d = nc.dram_tensor("tailrow", [1, 20], f32, kind="ExternalInput")
    if general:
        extrarow_d = nc.dram_tensor("extrarow", [1, 4 * E], f32, kind="ExternalInput")
    out_d = nc.dram_tensor("out", [256, E], f32, kind="ExternalOutput")

    def bcast(ap, parts):
        return bass.AP(tensor=ap.tensor, offset=ap.offset, ap=[[0, parts]] + list(ap.ap))

    with tile.TileContext(nc) as tc:
        with (
            tc.tile_pool(name="consts", bufs=1) as consts,
            tc.tile_pool(name="work", bufs=1) as work,
            tc.tile_pool(name="small", bufs=2) as small,
            tc.tile_pool(name="psum", bufs=1, space="PSUM") as psum,
        ):
            # ---- PX/PY broadcast via chunked DMAs on both HWDGE queues ----
            PX = consts.tile([128, N], f32)
            PY = consts.tile([128, N], f32)
            CH = 4
            for c in range(CH):
                sl = slice(c * (128 // CH), (c + 1) * (128 // CH))
                eng = nc.sync if c % 2 == 0 else nc.scalar
                eng.dma_start(out=PX[sl, :], in_=bcast(rowflat_d[0, 0:N], 128 // CH))
            for c in range(CH):
                sl = slice(c * (128 // CH), (c + 1) * (128 // CH))
                eng = nc.sync if c % 2 == 0 else nc.scalar
                eng.dma_start(out=PY[sl, :], in_=bcast(rowflat_d[0, N : 2 * N], 128 // CH))
            COLCAT = [consts.tile([128, 6], f32, name=f"COLCAT{it}") for it in range(2)]
            nc.sync.dma_start(out=COLCAT[0], in_=colcat_d[0:128, :])
            nc.scalar.dma_start(out=COLCAT[1], in_=colcat_d[128:256, :])
            ROWFLAT = consts.tile([1, 6 * N], f32)
            nc.sync.dma_start(out=ROWFLAT[:, 2 * N :], in_=rowflat_d[:, 2 * N :])
            TAILROW = consts.tile([1, 20], f32)
            nc.sync.dma_start(out=TAILROW, in_=tailrow_d[:, :])
            WV16 = consts.tile([16, E], f32)
            nc.scalar.dma_start(out=WV16, in_=wv16_d[:, :])
            W1S = consts.tile([E, E], f32)
            nc.scalar.dma_start(out=W1S, in_=w1_d[:, :])
            W2S = consts.tile([E, E], f32)
            nc.scalar.dma_start(out=W2S, in_=w2_d[:, :])
            if general:
                EXTRAROW = consts.tile([1, 4 * E], f32)
                nc.sync.dma_start(out=EXTRAROW, in_=extrarow_d[:, :])

            # ---- Act Sqrt table preload (dummy) + bias consts ----
            dum1 = consts.tile([128, 1], f32)
            nc.gpsimd.memset(dum1, 1.0)
            EPS_T = consts.tile([128, 1], f32)
            nc.gpsimd.memset(EPS_T, float(EPS))
            LNEPS_T = consts.tile([128, 1], f32)
            nc.gpsimd.memset(LNEPS_T, float(LN_EPS))
            CXB_T = consts.tile([128, 1], f32)
            nc.gpsimd.memset(CXB_T, float(CX_BIAS))
            dumo = consts.tile([128, 1], f32)
            nc.scalar.activation(dumo, dum1, Act.Sqrt)

            # ---- WR / tail consts broadcast on gpsimd ----
            WR = consts.tile([128, H, N], f32)
            for h in range(H):
                nc.gpsimd.partition_broadcast(
                    WR[:, h, :], ROWFLAT[0:1, (2 + h) * N : (3 + h) * N]
                )
            TAILC = consts.tile([128, 20], f32)
            nc.gpsimd.partition_broadcast(TAILC, TAILROW[0:1, :])
            ZC = TAILC[:, 0:4]
            SC = TAILC[:, 4:20]
            if general:
                EXTRAC = consts.tile([128, 4 * E], f32)
                nc.gpsimd.partition_broadcast(EXTRAC, EXTRAROW[0:1, :])
                B1R = EXTRAC[:, 0:E]
                GR = EXTRAC[:, E : 2 * E]
                BR = EXTRAC[:, 2 * E : 3 * E]
                B2R = EXTRAC[:, 3 * E : 4 * E]
            IDENT = consts.tile([128, 128], f32)
            make_identity(nc, IDENT)

            pcol0 = [COLCAT[it][:, 0:1] for it in range(2)]
            pcol1 = [COLCAT[it][:, 1:2] for it in range(2)]
            wcol = [[COLCAT[it][:, 2 + h : 3 + h] for h in range(H)] for it in range(2)]

            # ---- features: it-interleaved so Act stages batch ----
            def wtile(nm, it, dt=f32):
                return work.tile([128, N], dt, tag=f"{nm}{it}", name=f"{nm}{it}")

            dx, dy, dx2, dy2, r2, dist, inv, ld, cx, cy = ({} for _ in range(10))
            for it in range(2):
                dx[it] = wtile("dx", it)
                nc.vector.tensor_scalar_sub(dx[it], PX, pcol0[it])
                dy[it] = wtile("dy", it)
                nc.vector.tensor_scalar_sub(dy[it], PY, pcol1[it])
            for it in range(2):
                dx2[it] = wtile("dx2", it)
                nc.vector.tensor_mul(dx2[it], dx[it], dx[it])
                dy2[it] = wtile("dy2", it)
                nc.vector.tensor_mul(dy2[it], dy[it], dy[it])
            for it in range(2):
                r2[it] = wtile("r2", it)
                nc.vector.tensor_add(r2[it], dx2[it], dy2[it])
            for it in range(2):
                dist[it] = wtile("dist", it)
                nc.scalar.activation(dist[it], r2[it], Act.Sqrt, bias=EPS_T[:, :])
            for it in range(2):
                inv[it] = wtile("inv", it)
                nc.vector.reciprocal_approx_fast(out=inv[it], in_=dist[it])
            for it in range(2):
                cx[it] = wtile("cx", it)
                nc.vector.tensor_mul(cx[it], dx[it], inv[it])
                cy[it] = wtile("cy", it)
                nc.vector.tensor_mul(cy[it], dy[it], inv[it])
            # Ln block: ld (f32) + replay log-planes (bf16)
            lncx, lncy, lnld = {}, {}, {}
            for it in range(2):
                ld[it] = wtile("ld", it)
                nc.scalar.activation(ld[it], dist[it], Act.Ln, bias=1.0)
            if REPLAY_CXCY:
                for it in range(2):
                    lncx[it] = wtile("lncx", it, bf16)
                    nc.scalar.activation(lncx[it], cx[it], Act.Ln, bias=CXB_T[:, :])
                    lncy[it] = wtile("lncy", it, bf16)
                    nc.scalar.activation(lncy[it], cy[it], Act.Ln, bias=CXB_T[:, :])
            if REPLAY_LD:
                for it in range(2):
                    lnld[it] = wtile("lnld", it, bf16)
                    nc.scalar.activation(lnld[it], ld[it], Act.Ln, scale=LD_SCALE)

            # ---- logits + exp (E in bf16; l2 in bf16 for replay adds) ----
            Z, Es, l2b = {}, {}, {}
            junk = [
                work.tile([128, N], bf16, tag=f"junk{i}", name=f"junk{i}") for i in range(2)
            ]
            Sd, Sa = {}, {}
            # replay products: DVE bf16 add + Act exp-accumulate
            rcount = [0]

            def replay(it, h, lnplane, col):
                addp = work.tile([128, N], bf16, tag="addp", name="addp", bufs=4)
                nc.vector.tensor_add(addp, l2b[it][h], lnplane)
                jt = junk[rcount[0] % 2]
                rcount[0] += 1
                nc.scalar.activation(jt, addp, Act.Exp, accum_out=Sa[it][:, col : col + 1])

            for it in range(2):
                Z[it] = small.tile([128, H], f32, tag=f"Z{it}", name=f"Z{it}")
                Sd[it] = small.tile([128, 16], f32, tag=f"Sd{it}", name=f"Sd{it}")
                nc.gpsimd.memset(Sd[it], 0.0)
                Sa[it] = small.tile([128, 16], f32, tag=f"Sa{it}", name=f"Sa{it}")
                nc.gpsimd.memset(Sa[it], 0.0)
                Es[it] = []
                l2b[it] = []
            for it in range(2):
                for h in range(H):
                    x = work.tile([128, N], f32, tag="x", name="x", bufs=2)
                    nc.vector.scalar_tensor_tensor(
                        x, WR[:, h, :], wcol[it][h], inv[it], op0=Op.subtract, op1=Op.mult
                    )
                    l1 = work.tile([128, N], f32, tag="l1", name="l1", bufs=2)
                    nc.vector.scalar_tensor_tensor(
                        l1, dist[it], a0[h], x, op0=Op.mult, op1=Op.add
                    )
                    l2 = work.tile([128, N], bf16, tag=f"l2_{h}_{it}", name=f"l2_{h}_{it}")
                    nc.vector.scalar_tensor_tensor(
                        l2, ld[it], a3[h], l1, op0=Op.mult, op1=Op.add
                    )
                    l2b[it].append(l2)
                    Eh = work.tile([128, N], bf16, tag=f"E{h}_{it}", name=f"E{h}_{it}")
                    nc.scalar.activation(Eh, l2, Act.Exp, accum_out=Z[it][:, h : h + 1])
                    Es[it].append(Eh)
                    # interleave this head's S-products right here
                    if REPLAY_CXCY:
                        replay(it, h, lncx[it], h * 4 + 1)
                        replay(it, h, lncy[it], h * 4 + 2)
                    else:
                        for p, feat in ((1, cx[it]), (2, cy[it])):
                            prod = work.tile([128, N], bf16, tag="prod", name="prod", bufs=2)
                            nc.vector.scalar_tensor_tensor(
                                prod, Es[it][h], 1.0, feat, op0=Op.mult, op1=Op.mult,
                                accum_out=Sd[it][:, h * 4 + p : h * 4 + p + 1],
                            )
                    if REPLAY_LD:
                        replay(it, h, lnld[it], h * 4 + 3)
                    else:
                        prod = work.tile([128, N], bf16, tag="prod", name="prod", bufs=2)
                        nc.vector.scalar_tensor_tensor(
                            prod, Es[it][h], 1.0, ld[it], op0=Op.mult, op1=Op.mult,
                            accum_out=Sd[it][:, h * 4 + 3 : h * 4 + 4],
                        )
                    prod = work.tile([128, N], bf16, tag="prod", name="prod", bufs=2)
                    nc.vector.scalar_tensor_tensor(
                        prod, Es[it][h], 1.0, dist[it], op0=Op.mult, op1=Op.mult,
                        accum_out=Sd[it][:, h * 4 : h * 4 + 1],
                    )

            # ---- per-tile tail ----
            for it in range(2):
                Zc = small.tile([128, H], f32, tag=f"Zc{it}", name=f"Zc{it}")
                nc.vector.tensor_sub(Zc, Z[it], ZC)
                Zi = small.tile([128, H], f32, tag=f"Zi{it}", name=f"Zi{it}")
                nc.vector.reciprocal(Zi, Zc)
                Ssum = small.tile([128, 16], f32, tag=f"Ssum{it}", name=f"Ssum{it}")
                nc.vector.tensor_add(Ssum, Sd[it], Sa[it])
                Sn = small.tile([128, 16], f32, tag=f"Sn{it}", name=f"Sn{it}")
                nc.vector.tensor_sub(Sn, Ssum, SC)
                if REPLAY_CXCY:
                    # cx/cy columns: subtract runtime 2*Zc
                    for h in range(H):
                        for p in (1, 2):
                            c0 = h * 4 + p
                            nc.vector.scalar_tensor_tensor(
                                Sn[:, c0 : c0 + 1], Zc[:, h : h + 1], -2.0,
                                Sn[:, c0 : c0 + 1], op0=Op.mult, op1=Op.add,
                            )
                for h in range(H):
                    nc.vector.tensor_scalar_mul(
                        Sn[:, h * 4 : h * 4 + 4], Sn[:, h * 4 : h * 4 + 4], Zi[:, h : h + 1]
                    )
                ps_t = psum.tile([16, 128], f32, tag="ps_t", name="ps_t")
                nc.tensor.transpose(ps_t, Sn, IDENT)
                SNT = small.tile([16, 128], f32, tag=f"SNT{it}", name=f"SNT{it}")
                nc.vector.tensor_copy(SNT, ps_t)
                ctx_ps = psum.tile([128, 128], f32, tag="ctx_ps", name="ctx_ps")
                nc.tensor.matmul(ctx_ps, lhsT=WV16, rhs=SNT, start=True, stop=True)
                ctxT = small.tile([128, 128], f32, tag=f"ctxT{it}", name=f"ctxT{it}")
                nc.vector.tensor_copy(ctxT, ctx_ps)
                h1_ps = psum.tile([128, E], f32, tag="h1", name="h1")
                nc.tensor.matmul(h1_ps, lhsT=ctxT, rhs=W1S, start=True, stop=True)
                if skip_b1:
                    h1b = h1_ps
                else:
                    h1b = small.tile([128, E], f32, tag=f"h1b{it}", name=f"h1b{it}")
                    nc.vector.tensor_add(h1b, h1_ps, B1R)
                stats = small.tile([128, 6], f32, tag="stats", name="stats")
                nc.vector.bn_stats(stats, h1b)
                mv = small.tile([128, 2], f32, tag="mv", name="mv")
                nc.vector.bn_aggr(mv, stats)
                sd = small.tile([128, 1], f32, tag="sd", name="sd")
                nc.scalar.activation(sd, mv[:, 1:2], Act.Sqrt, bias=LNEPS_T[:, :])
                rstd = small.tile([128, 1], f32, tag="rstd", name="rstd")
                nc.vector.reciprocal(rstd, sd)
                xc = small.tile([128, E], f32, tag="xc", name="xc")
                nc.vector.tensor_scalar(
                    xc, h1b, scalar1=mv[:, 0:1], scalar2=rstd, op0=Op.subtract, op1=Op.mult
                )
                y2 = xc
                if not skip_ln:
                    y1 = small.tile([128, E], f32, tag="y1", name="y1")
                    nc.vector.tensor_mul(y1, xc, GR)
                    y2 = small.tile([128, E], f32, tag="y2", name="y2")
                    nc.vector.tensor_add(y2, y1, BR)
                g = small.tile([128, E], f32, tag="g", name="g")
                nc.scalar.activation(g, y2, Act.Gelu)
                g_ps = psum.tile([128, 128], f32, tag="g_ps", name="g_ps")
                nc.tensor.transpose(g_ps, g, IDENT)
                gT = small.tile([128, 128], f32, tag="gT", name="gT")
                nc.vector.tensor_copy(gT, g_ps)
                h2_ps = psum.tile([128, E], f32, tag="h2", name="h2")
                nc.tensor.matmul(h2_ps, lhsT=gT, rhs=W2S, start=True, stop=True)
                outt = small.tile([128, E], f32, tag=f"outt{it}", name=f"outt{it}")
                if skip_b2:
                    nc.vector.tensor_copy(outt, h2_ps)
                else:
                    nc.vector.tensor_add(outt, h2_ps, B2R)
                eng = nc.sync if it == 0 else nc.scalar
                eng.dma_start(out=out_d[ts(it, 128), :], in_=outt)

    nc.compile()
    return nc


last_results = None


def kernel(positions, key_padding_mask, kv_w, kv_b, query, w1, b1, ln_g, ln_b, w2, b2):
    from concourse.bass_utils import run_bass_kernel_spmd

    per_core, A, flags = _host_prep(
        positions, key_padding_mask, kv_w, kv_b, query, w1, b1, ln_g, ln_b, w2, b2
    )
    nc = _build_program(A, flags)
    res = run_bass_kernel_spmd(nc, per_core, core_ids=list(range(NCORES)))
    global last_results
    last_results = res
    out = np.empty((B, N, E), dtype=np.float32)
    for c in range(NCORES):
        b = c // 2
        i0 = (c % 2) * 256
        out[b, i0 : i0 + 256] = res.results[c]["out"]
    return out
